# revision 41
# baseline (speedup 1.0000x reference)
"""Multi-head attention kernel for Trainium2, tensor-parallel over heads on 8 cores.

Strategy (per core c, heads [2c, 2c+1]):
  - host feeds X^T [D, B*S] (shared), per-core transposed head weights, and the
    matching Wo column-slice; each core computes a full-shape partial of the
    output projection (fp16), host sums the 8 partials and adds bo.
  - on device everything is computed in "transposed" orientation so every
    matmul contracts over the partition dim (only V needs a PE-transpose).
    All matmul operands are fp16, accumulation stays fp32:
      QT/KT/VT [e, s] = W @ X^T          (fp16 matmuls, N=512)
      S^T [t, s]      = KT.T @ QT        (per (b, head), C=64, head pair
                                          row-group-packed and concurrent)
      P^T             = exp(S^T / 8)     (ACT, PSUM->SBUF; the wall: ~284us)
      [avT ; l]       = [V | 1].T @ P^T  (fused unnormalized attention + sum)
      bc              = 1s.T @ (1/l)     (C=1 PE broadcast matmul, fp32r)
      Z               = avT * bc         (DVE)
      out_partial     = Z.T @ WoT_slice  (PSUM -> SBUF fp16 -> DRAM)

  Scheduling (the Tile scheduler orders each engine's queue by emission
  priority among dependency-ready instructions, so program order is the
  pacing tool):
    - the ACT exp stream (~1.11us per t-chunk pair) and the PE are both
      near-saturated mid-tile; all projection / out-projection work is
      split into <=4-matmul work items in a build-time queue, popped 2-3
      per t-iteration so it interleaves with (never preempts) the
      exp-critical score matmuls;
    - each s-tile's last 3 AV pairs + PSUM drain (z copies, 1/l, DRAM-
      bounce broadcast, normalize) are carried into the next s-tile's
      first iterations, so s-tile boundaries stay seamless;
    - out-projection of s-tile n is deferred into s-tile n+1 (hides the
      broadcast latency); the final s-tile normalizes via a C=1 PE
      broadcast matmul instead of the DRAM bounce (shorter kernel tail);
    - batch 0 loads x before weights (bias DMAs first), warms the PE HAM
      clock with junk matmuls during the x DMA, and feeds K/Q for s-tile 0
      only, so the exp stream starts ~21us in; junk matmuls at batch seams
      keep the clock at 2.4GHz through the drain chains.
  Baseline 465.6us -> ~414-422us measured (rel err 5.6e-4); each batch carries its own V-tail (V st3 + trans 8-15) in its first s-tile rather than crowding the previous batch's window.
"""

import numpy as np

import concourse.bass as bass
import concourse.mybir as mybir
import concourse.tile as tile
from concourse import bacc
from concourse.bass_utils import run_bass_kernel_spmd
from concourse.masks import make_identity

# Problem shapes (hardcoded per contract).
B, S, D = 4, 2048, 1024
H, E = 16, 64
NCORES = 8
HPC = H // NCORES          # heads per core = 2
EC = HPC * E               # per-core head width = 128
BS = B * S                 # 8192 rows
P = 128
DC = D // P                # 8 contraction chunks for the projections
DC2 = DC // 2              # 4 double-row chain steps
ST = 512                   # s tile (matmul moving free dim)
N_ST = S // ST             # 4 s-tiles per batch
TCH = S // P               # 16 key chunks per batch

F32 = mybir.dt.float32
F32R = mybir.dt.float32r
F16 = mybir.dt.float16
F8 = mybir.dt.float8e4
DR = mybir.MatmulPerfMode.DoubleRow
EXP = mybir.ActivationFunctionType.Exp

# Host-side scaling: weights are multiplied by WSCALE before the fp8e4 cast
# so their magnitude (~1/32) lands in e4m3's sweet spot.  Q and K then carry
# WSCALE each, so the exp argument is scores/(8*WSCALE^2); V carries WSCALE,
# compensated by dividing Wo by WSCALE on the host.  The exp bias -ln(32)
# keeps the unnormalized z accumulation inside fp16 range (it cancels in the
# softmax normalization).
WSCALE = 32.0
EXP_SCALE = 0.125 / (WSCALE * WSCALE)
EXP_BIAS = -3.4657359027997265  # -ln(32)


def _r(ap):
    return ap.bitcast(F32R)


def build_module():
    """Build the single-core Bass module (same NEFF runs SPMD on all 8 cores)."""
    from contextlib import ExitStack

    nc = bacc.Bacc("TRN2", target_bir_lowering=False, debug=False)
    xt = nc.dram_tensor("xt", [D, BS], F16, kind="ExternalInput").ap()
    wq = nc.dram_tensor("wq_t", [D, EC], F16, kind="ExternalInput").ap()
    wk = nc.dram_tensor("wk_t", [D, EC], F16, kind="ExternalInput").ap()
    wv = nc.dram_tensor("wv_t", [D, EC], F16, kind="ExternalInput").ap()
    bq = nc.dram_tensor("bq", [EC, 1], F32, kind="ExternalInput").ap()
    bk = nc.dram_tensor("bk", [EC, 1], F32, kind="ExternalInput").ap()
    bv = nc.dram_tensor("bv", [EC, 1], F32, kind="ExternalInput").ap()
    wo = nc.dram_tensor("wo_t", [EC, D], F16, kind="ExternalInput").ap()
    outp = nc.dram_tensor("out_p", [BS, D], F16, kind="ExternalOutput").ap()

    xt_r = xt.rearrange("(dc p) s -> p dc s", p=P)    # [128, 8, 8192]
    wq_r = wq.rearrange("(dc p) e -> p dc e", p=P)    # [128, 8, 128]
    wk_r = wk.rearrange("(dc p) e -> p dc e", p=P)
    wv_r = wv.rearrange("(dc p) e -> p dc e", p=P)

    with tile.TileContext(nc) as tc, ExitStack() as ctx:
        singles = ctx.enter_context(tc.tile_pool(name="singles", bufs=1))

        wq_sb = singles.tile([P, DC, EC], F16, tag="wq")
        wk_sb = singles.tile([P, DC, EC], F16, tag="wk")
        wv_sb = singles.tile([P, DC, EC], F16, tag="wv")
        bq_sb = singles.tile([EC, 1], F32, tag="bq")
        bk_sb = singles.tile([EC, 1], F32, tag="bk")
        bv_sb = singles.tile([EC, 1], F32, tag="bv")
        wo_sb = singles.tile([EC, D], F16, tag="wo")
        ident = singles.tile([P, P], F16, tag="ident")
        make_identity(nc, ident[:])
        ones16 = singles.tile([1, E], F16, tag="ones16")
        nc.vector.memset(ones16[:], 1.0)
        ebias = singles.tile([P, 1], F32, tag="ebias")
        nc.vector.memset(ebias[:], EXP_BIAS)

        # Per-batch persistent activations: [e, s] projections and V_ext.
        qt = [singles.tile([EC, S], F16, tag=f"qt{b}", name=f"qt{b}") for b in range(B)]
        kt = [singles.tile([EC, S], F16, tag=f"kt{b}", name=f"kt{b}") for b in range(B)]
        vt = [singles.tile([EC, S], F16, tag=f"vtz{b}", name=f"vt{b}") for b in range(B)]
        # V_ext layout: [t-part, t-chunk, 130] = [V_h0 | 1 | V_h1 | 1]
        vx = [singles.tile([P, TCH, 2 * E + 2], F16, tag=f"vx{b}", name=f"vx{b}") for b in range(B)]
        for b in range(B):
            nc.vector.memset(vx[b][:, :, E : E + 1], 1.0)
            nc.vector.memset(vx[b][:, :, 2 * E + 1 : 2 * E + 2], 1.0)

        z = [singles.tile([EC, S], F16, tag=f"z{b}", name=f"z{b}") for b in range(B)]
        xts = [[None] * N_ST for _ in range(B)]
        CRIT = -2000000  # junk clock-warmers: run only when PE is otherwise idle
        # Priority classes for queued bulk work.  The Tile scheduler is a
        # timing simulation that, whenever an engine goes idle, runs the
        # highest-priority *dependency-ready* instruction.  Emission order
        # still defines the dependency DAG (tile-slot reuse), but by emitting
        # bulk work at low priority the critical score->exp->AV stream jumps
        # ahead of any backlog the moment its own inputs are ready, instead
        # of queueing behind 4+ projection chains at every s-tile boundary.
        # Deadline-ordered tiers: next-batch K/V/x/transposes are ALL
        # consumed within that batch's FIRST s-tile, so they are due at the
        # seam; the out-projection has multiple tiles of staging slack.
        SOON = 300000    # qt next tile, V tail, and next-batch K/V/x/trans
        BULK = 450000    # out-projection (runs in later PE gaps)
        PBIAS = 250000   # proj bias-adds: they release shared PSUM slots, so
                         # they outrank all queued bulk work on the DVE heap

        with (
            tc.tile_pool(name="xload", bufs=6) as xpool,
            tc.tile_pool(name="pexp", bufs=14) as ppool,
            tc.tile_pool(name="bcast", bufs=3) as bpool,
            tc.tile_pool(name="ostage", bufs=6) as opool,
            tc.tile_pool(name="lrow", bufs=2, space="DRAM") as dpool,
            tc.tile_pool(name="psum", bufs=2, space="PSUM") as psum,
            tc.tile_pool(name="psum_av", bufs=1, space="PSUM") as psum_av,
        ):

            from collections import deque

            gap_q = deque()  # entries: (prio, seq, fn); FIFO mid-tile pops
            gap_seq = [0]

            def gap_push(prio, fn):
                gap_q.append((prio, gap_seq[0], fn))
                gap_seq[0] += 1

            def pop_gap():
                prio, _, fn = gap_q.popleft()
                with tc.high_priority(offset=-prio):
                    fn()

            def drain_gap_by_prio():
                # At s-tile entry the whole backlog must be emitted before the
                # tile's scores (emission order defines the dependency DAG).
                # Emit it deadline-first: the PSUM pool slot rotation is fixed
                # by emission order, so this keeps seam-critical K/V chains
                # from queueing behind out-projection slots that are held by
                # slow DVE casts.
                for prio, _, fn in sorted(gap_q, key=lambda e: (e[0], e[1])):
                    with tc.high_priority(offset=-prio):
                        fn()
                gap_q.clear()

            def _bulk(fn):
                with tc.high_priority(offset=-SOON):
                    fn()

            def emit_x(b, sts=range(N_ST), fine=False):
                # split each 1MB tile into 4 DMAs so they spread across
                # queues; `fine` splits per dc-chunk so the batch-0 head's
                # K projection chain can start on partial data
                for st in sts:
                    g = b * N_ST + st
                    x_t = xpool.tile([P, DC, ST], F16, tag="xt", name="x_t")
                    step = 1 if fine else 2
                    for hh in range(DC // step):
                        nc.sync.dma_start(
                            x_t[:, step * hh : step * hh + step],
                            xt_r[:, step * hh : step * hh + step,
                                 g * ST : (g + 1) * ST],
                        )
                    xts[b][st] = x_t

            def emit_proj(b, st, w_sb, b_sb, dst):
                # One 512-wide projection chain: dst[:, st] = W @ X^T + bias
                sl = slice(st * ST, (st + 1) * ST)
                ps = psum.tile([P, ST], F32, tag="mm", name="ps")
                for dc in range(DC):
                    nc.tensor.matmul(
                        ps[:], w_sb[:, dc], xts[b][st][:, dc],
                        start=(dc == 0), stop=(dc == DC - 1),
                    )
                nc.vector.tensor_scalar_add(dst[:, sl], ps[:], b_sb[:])

            def push_proj(b, st, w_sb, b_sb, dst, prio=SOON):
                # Same chain, split into two 2-matmul work items: small enough
                # to interleave with the exp stream, big enough to keep the
                # PSUM-slot hold time short.
                sl = slice(st * ST, (st + 1) * ST)
                box = {}

                def item(i):
                    def go():
                        if i == 0:
                            box["ps"] = psum.tile([P, ST], F32, tag="mm", name="ps")
                        ps = box["ps"]
                        for dc in range(4 * i, 4 * i + 4):
                            nc.tensor.matmul(
                                ps[:], w_sb[:, dc], xts[b][st][:, dc],
                                start=(dc == 0), stop=(dc == DC - 1),
                            )
                        if i == 1:
                            with tc.high_priority(offset=prio - PBIAS):
                                nc.vector.tensor_scalar_add(dst[:, sl], ps[:], b_sb[:])
                    return go

                for i in range(2):
                    gap_push(prio, item(i))

            def emit_trans(b, chunks):
                # PE-transpose batch b's V chunks into V_ext.
                for tch in chunks:
                    tp = psum.tile([P, ST], F16, tag="mm", name="tp")
                    nc.tensor.transpose(
                        tp[:, 0:P], vt[b][:, tch * P : (tch + 1) * P], ident[:]
                    )
                    nc.vector.tensor_copy(vx[b][:, tch, 0:E], tp[:, 0:E])
                    nc.vector.tensor_copy(
                        vx[b][:, tch, E + 1 : 2 * E + 1], tp[:, E : 2 * E]
                    )

            def push_trans(b, lo, hi, prio=SOON):
                for c0 in range(lo, hi, 2):
                    gap_push(prio, lambda c0=c0: emit_trans(b, range(c0, c0 + 2)))

            def push_outproj(b, st, final=False):
                for c in range(ST // P):
                    def go(c=c):
                        zsl = slice(st * ST + c * P, st * ST + (c + 1) * P)
                        rows = slice(b * S + st * ST + c * P,
                                     b * S + st * ST + (c + 1) * P)
                        for oh in range(D // 512):
                            po = psum.tile([P, ST], F32, tag="mm", name="po")
                            nc.tensor.matmul(
                                po[:], z[b][:, zsl],
                                wo_sb[:, oh * 512 : (oh + 1) * 512],
                                start=True, stop=True,
                            )
                            osb = opool.tile([P, 512], F16, tag="osb", name="osb")
                            if final and (c + oh) % 2 == 1:
                                # kernel tail: ACT is idle, split the casts
                                # across both engines to drain PSUM 2x faster
                                nc.scalar.copy(osb[:], po[:])
                            else:
                                nc.vector.tensor_copy(osb[:], po[:])
                            if final and c % 2 == 1:
                                nc.scalar.dma_start(
                                    outp[rows, oh * 512 : (oh + 1) * 512], osb[:])
                            else:
                                nc.sync.dma_start(
                                    outp[rows, oh * 512 : (oh + 1) * 512], osb[:])
                    gap_push(BULK, go)

            pending = [None]
            carrybox = {"c": []}
            AVLAG = 3

            def emit_attn(b, st, hooks):
                # batch seams tend to idle the PE >3.4us on the DVE drain
                # chain, dropping HAM to 1.2GHz right before a burst of
                # deferred chains — junk matmuls keep the clock up.
                if st == 0 and b > 0:
                    with tc.high_priority(offset=CRIT):
                        for _ in range(10):
                            nc.tensor.matmul(wu[:, 0:P], ident[:], ident[:],
                                             start=True, stop=True)
                # safety: queued producers for this s-tile's inputs must be
                # EMITTED before its consumers (emission order defines the
                # dependency DAG).  They are emitted at their low priority, so
                # this s-tile's scores still run ahead of any backlog that
                # this tile's inputs don't actually depend on.
                drain_gap_by_prio()
                ssl = slice(st * ST, (st + 1) * ST)
                av = psum_av.tile([P, 2, ST], F32, tag="av", name="av")
                av0 = av[:, 0]
                av1 = av[:, 1]
                pts = {}

                def av_pair(t):
                    p = pts.pop(t)
                    nc.tensor.matmul(
                        av0[0 : E + 1], vx[b][:, t, 0 : E + 1], p[:, 0],
                        start=(t == 0), stop=(t == TCH - 1),
                    )
                    nc.tensor.matmul(
                        av1[0 : E + 1], vx[b][:, t, E + 1 : 2 * E + 2], p[:, 1],
                        start=(t == 0), stop=(t == TCH - 1),
                    )

                def drain():
                    # unnormalized copy out of PSUM (frees the av banks),
                    # reciprocal of the fused row-sums, broadcast, normalize.
                    # The l rows are copied to TWO partitions so the serial
                    # reciprocal runs at FD=512 instead of FD=1024.
                    nc.vector.tensor_copy(z[b][0:E, ssl], av0[0:E])
                    nc.vector.tensor_copy(z[b][E : 2 * E, ssl], av1[0:E])
                    lr = bpool.tile([1, 2, ST], F32, tag="lr", name="lr")
                    nc.vector.tensor_copy(lr[0:1], av[E : E + 1, :, :])
                    nc.vector.reciprocal_approx_fast(out=lr[0:1], in_=lr[0:1])
                    if (b, st) == (B - 1, N_ST - 1):
                        # final s-tile: a DRAM bounce would sit on the kernel
                        # tail; use C=1 PE broadcast matmuls instead (moving
                        # must sit at partition 0, so cast each head's row
                        # down separately).
                        lr16a = bpool.tile([1, ST], F16, tag="lr16a", name="lr16a")
                        lr16b = bpool.tile([1, ST], F16, tag="lr16b", name="lr16b")
                        nc.vector.tensor_copy(lr16a[0:1], lr[0:1, 0, :])
                        nc.vector.tensor_copy(lr16b[0:1], lr[0:1, 1, :])
                        bc0 = psum.tile([E, ST], F32, tag="mm", name="bc0")
                        nc.tensor.matmul(bc0[:], ones16[0:1, :], lr16a[0:1, :],
                                         start=True, stop=True)
                        nc.vector.tensor_mul(z[b][0:E, ssl], z[b][0:E, ssl], bc0[:])
                        bc1 = psum.tile([E, ST], F32, tag="mm", name="bc1")
                        nc.tensor.matmul(bc1[:], ones16[0:1, :], lr16b[0:1, :],
                                         start=True, stop=True)
                        nc.vector.tensor_mul(
                            z[b][E : 2 * E, ssl], z[b][E : 2 * E, ssl], bc1[:])
                    else:
                        lrow = dpool.tile([2, ST], F32, tag="lrow", name="lrow")
                        nc.sync.dma_start(
                            bass.AP(tensor=lrow.tensor, offset=lrow.offset,
                                    ap=[[0, 1]] + list(lrow.ap)),
                            lr[0:1, :, :],
                        )
                        bc = bpool.tile([P, ST], F32, tag="bc", name="bc")
                        nc.sync.dma_start(
                            bc[0:E],
                            bass.AP(tensor=lrow.tensor, offset=lrow.offset,
                                    ap=[[0, E]] + list(lrow[0, :].ap)),
                        )
                        nc.sync.dma_start(
                            bc[E : 2 * E],
                            bass.AP(tensor=lrow.tensor, offset=lrow.offset + ST,
                                    ap=[[0, E]] + list(lrow[1, :].ap)),
                        )
                        nc.vector.tensor_mul(
                            z[b][0:E, ssl], z[b][0:E, ssl], bc[0:E])
                        nc.vector.tensor_mul(
                            z[b][E : 2 * E, ssl], z[b][E : 2 * E, ssl], bc[E : 2 * E]
                        )

                prev = carrybox["c"]
                for t in range(TCH):
                    tsl = slice(t * P, (t + 1) * P)
                    sc = psum.tile([P, 2, ST], F32, tag="sc", name="sc")
                    nc.tensor.matmul(
                        sc[:, 0], kt[b][0:E, tsl], qt[b][0:E, ssl],
                        start=True, stop=True,
                    )
                    nc.tensor.matmul(
                        sc[:, 1], kt[b][E : 2 * E, tsl], qt[b][E : 2 * E, ssl],
                        start=True, stop=True,
                    )
                    pt = ppool.tile([P, 2, ST], F16, tag="pt", name="pt")
                    pts[t] = pt
                    nc.scalar.activation(pt[:], sc[:], EXP, scale=0.125)
                    for fn in hooks.get(t, ()):
                        fn()
                    if t == 4 and pending[0] is not None:
                        push_outproj(*pending[0])
                        pending[0] = None
                    # previous s-tile's last av pairs + drain, carried into
                    # this s-tile's first iters (keeps boundaries seamless)
                    if t < len(prev):
                        prev[t]()
                    if t >= AVLAG:
                        av_pair(t - AVLAG)
                    for _ in range(3 if len(gap_q) > 12 else 2):
                        if gap_q:
                            pop_gap()
                carrybox["c"] = [
                    lambda: av_pair(TCH - 3),
                    lambda: av_pair(TCH - 2),
                    lambda: (av_pair(TCH - 1), drain()),
                ]
                pending[0] = (b, st)

            # ---- push schedules: all projection / transpose / prefetch work
            # for the next batch rides the work queue, paced 2 items/iter.
            # Producers must be QUEUED before their consumers are emitted
            # (program order defines dependencies).
            def hooks_for(b, st):
                hk = {}

                def add(t, fn):
                    hk.setdefault(t, []).append(fn)

                if st + 1 < N_ST:
                    add(0, lambda: push_proj(b, st + 1, wq_sb, bq_sb, qt[b], prio=SOON))
                if st == 0:
                    # this batch's own V tail: consumed only at iters 11+,
                    # so it can ride this batch's own first s-tile instead
                    # of crowding the previous batch's window.
                    add(0, lambda: push_proj(b, 3, wv_sb, bv_sb, vt[b], prio=SOON))
                    add(2, lambda: push_trans(b, 8, 12, prio=SOON))
                    add(4, lambda: push_trans(b, 12, 16, prio=SOON))
                nb = b + 1
                if nb < B and b > 0:
                    if st == 0:
                        add(6, lambda: _bulk(lambda: emit_x(nb)))
                        add(8, lambda: push_proj(nb, 0, wk_sb, bk_sb, kt[nb]))
                        add(12, lambda: push_proj(nb, 1, wk_sb, bk_sb, kt[nb]))
                    elif st == 1:
                        add(4, lambda: push_proj(nb, 2, wk_sb, bk_sb, kt[nb]))
                        add(8, lambda: push_proj(nb, 3, wk_sb, bk_sb, kt[nb]))
                        add(12, lambda: push_proj(nb, 0, wv_sb, bv_sb, vt[nb]))
                    elif st == 2:
                        add(2, lambda: push_proj(nb, 1, wv_sb, bv_sb, vt[nb]))
                        add(6, lambda: push_trans(nb, 0, 4))
                        add(10, lambda: push_trans(nb, 4, 8))
                    else:
                        add(2, lambda: push_proj(nb, 0, wq_sb, bq_sb, qt[nb]))
                        add(6, lambda: push_proj(nb, 2, wv_sb, bv_sb, vt[nb]))
                return hk

            # ---- batch 0 head: tiny bias/weight DMAs first, x split across
            # queues, a PE warmup burst (flips HAM to 2.4GHz), then just
            # K0/Q0 so the exp stream starts ASAP.
            nc.sync.dma_start(bq_sb[:], bq)
            nc.sync.dma_start(bk_sb[:], bk)
            nc.sync.dma_start(bv_sb[:], bv)
            nc.sync.dma_start(wk_sb[:], wk_r)
            nc.sync.dma_start(wq_sb[:], wq_r)
            emit_x(0, sts=[0])
            nc.sync.dma_start(wv_sb[:], wv_r)
            nc.sync.dma_start(wo_sb[:], wo)
            emit_x(0, sts=[1, 2, 3])
            with tc.high_priority(offset=CRIT):
                wu = psum.tile([P, ST], F32, tag="mm", name="wu")
                for _ in range(28):
                    nc.tensor.matmul(wu[:, 0:P], ident[:], ident[:],
                                     start=True, stop=True)
            emit_proj(0, 0, wk_sb, bk_sb, kt[0])
            emit_proj(0, 0, wq_sb, bq_sb, qt[0])

            # batch-0 st0: V chunk 0-3 path emitted directly (av(0) needs it
            # almost immediately); the rest is queued in deadline order.
            b0_hooks = [
                {
                    0: [lambda: emit_proj(0, 0, wv_sb, bv_sb, vt[0]),
                        lambda: emit_trans(0, range(0, 2)),
                        lambda: push_proj(0, 1, wk_sb, bk_sb, kt[0], prio=SOON)],
                    1: [lambda: emit_trans(0, range(2, 4)),
                        lambda: push_proj(0, 1, wv_sb, bv_sb, vt[0], prio=SOON)],
                    2: [lambda: push_trans(0, 4, 8, prio=SOON),
                        lambda: push_proj(0, 2, wk_sb, bk_sb, kt[0], prio=SOON)],
                    5: [lambda: push_proj(0, 2, wv_sb, bv_sb, vt[0], prio=SOON),
                        lambda: push_trans(0, 8, 12, prio=SOON)],
                    7: [lambda: push_proj(0, 3, wk_sb, bk_sb, kt[0], prio=SOON)],
                    9: [lambda: push_proj(0, 3, wv_sb, bv_sb, vt[0], prio=SOON),
                        lambda: push_trans(0, 12, 16, prio=SOON)],
                    11: [lambda: push_proj(0, 1, wq_sb, bq_sb, qt[0], prio=SOON)],
                },
                {
                    0: [lambda: push_proj(0, 2, wq_sb, bq_sb, qt[0], prio=SOON)],
                    6: [lambda: _bulk(lambda: emit_x(1))],
                    8: [lambda: push_proj(1, 0, wk_sb, bk_sb, kt[1])],
                    12: [lambda: push_proj(1, 1, wk_sb, bk_sb, kt[1])],
                },
                {
                    0: [lambda: push_proj(0, 3, wq_sb, bq_sb, qt[0], prio=SOON)],
                    4: [lambda: push_proj(1, 2, wk_sb, bk_sb, kt[1])],
                    8: [lambda: push_proj(1, 3, wk_sb, bk_sb, kt[1])],
                    12: [lambda: push_proj(1, 0, wv_sb, bv_sb, vt[1])],
                },
                {
                    2: [lambda: push_proj(1, 1, wv_sb, bv_sb, vt[1])],
                    6: [lambda: push_trans(1, 0, 4)],
                    8: [lambda: push_proj(1, 0, wq_sb, bq_sb, qt[1])],
                    10: [lambda: push_proj(1, 2, wv_sb, bv_sb, vt[1])],
                    12: [lambda: push_trans(1, 4, 8)],
                },
            ]
            for st in range(N_ST):
                emit_attn(0, st, b0_hooks[st])

            for b in range(1, B):
                for st in range(N_ST):
                    emit_attn(b, st, hooks_for(b, st))

            # tail: leftover carry (last av pairs + final drain), remaining
            # queue, and the final s-tile's out-projection. Junk matmuls keep
            # the PE HAM-warm through the serial drain chain so the final
            # out-projection runs at 2.4GHz.
            for fn in carrybox["c"]:
                fn()
            with tc.high_priority(offset=CRIT):
                for _ in range(8):
                    nc.tensor.matmul(wu[:, 0:P], ident[:], ident[:],
                                     start=True, stop=True)
            while gap_q:
                pop_gap()
            push_outproj(*pending[0], final=True)
            while gap_q:
                pop_gap()
    nc.finalize()
    return nc


_NC_CACHE = None


def _get_module():
    global _NC_CACHE
    if _NC_CACHE is None:
        _NC_CACHE = build_module()
    return _NC_CACHE


def prepare_in_maps(inputs):
    x = np.ascontiguousarray(np.asarray(inputs["input_matrix"], np.float32))
    wq = np.asarray(inputs["Wq"], np.float32)
    wk = np.asarray(inputs["Wk"], np.float32)
    wv = np.asarray(inputs["Wv"], np.float32)
    bq = np.asarray(inputs["bq"], np.float32)
    bk = np.asarray(inputs["bk"], np.float32)
    bv = np.asarray(inputs["bv"], np.float32)
    wo = np.asarray(inputs["Wo"], np.float32)

    xt = np.ascontiguousarray(x.reshape(BS, D).T.astype(np.float16))  # [D, BS]
    in_maps = []
    for c in range(NCORES):
        hs = slice(HPC * c, HPC * (c + 1))
        m = {
            "xt": xt,
            "wq_t": np.ascontiguousarray(wq[hs].transpose(2, 0, 1).reshape(D, EC).astype(np.float16)),
            "wk_t": np.ascontiguousarray(wk[hs].transpose(2, 0, 1).reshape(D, EC).astype(np.float16)),
            "wv_t": np.ascontiguousarray(wv[hs].transpose(2, 0, 1).reshape(D, EC).astype(np.float16)),
            "bq": np.ascontiguousarray(bq[hs].reshape(EC, 1)),
            "bk": np.ascontiguousarray(bk[hs].reshape(EC, 1)),
            "bv": np.ascontiguousarray(bv[hs].reshape(EC, 1)),
            "wo_t": np.ascontiguousarray(wo[:, EC * c : EC * (c + 1)].T.astype(np.float16)),
        }
        in_maps.append(m)
    return in_maps


def finish(results, inputs):
    bo = np.asarray(inputs["bo"], np.float32)
    acc = results[0]["out_p"].astype(np.float32)
    for r in results[1:]:
        acc += r["out_p"].astype(np.float32)
    out = (acc + bo).astype(np.float32)
    return out.reshape(B, S, D)


def kernel(**inputs):
    nc = _get_module()
    in_maps = prepare_in_maps(inputs)
    res = run_bass_kernel_spmd(nc, in_maps, core_ids=list(range(NCORES)))
    return finish(res.results, inputs)


if __name__ == "__main__":
    import reference

    inputs = {k: np.asarray(v) for k, v in reference.setup_inputs().items()}
    out = kernel(**inputs)
    print(out.shape, out.dtype)



# revision 42
# speedup vs baseline: 1.0892x; 1.0892x over previous
"""Multi-head attention kernel for Trainium2, tensor-parallel over heads on 8 cores.

Strategy (per core c, heads [2c, 2c+1]):
  - host feeds X^T [D, B*S] (shared), per-core transposed head weights, and the
    matching Wo column-slice; each core computes a full-shape partial of the
    output projection (fp16), host sums the 8 partials and adds bo.
  - on device everything is computed in "transposed" orientation so every
    matmul contracts over the partition dim (only V needs a PE-transpose).
    All matmul operands are fp16, accumulation stays fp32:
      QT/KT/VT [e, s] = W @ X^T          (fp16 matmuls, N=512)
      S^T [t, s]      = KT.T @ QT        (per (b, head), C=64, head pair
                                          row-group-packed and concurrent)
      P^T             = exp(S^T / 8)     (ACT, PSUM->SBUF; the wall: ~284us)
      [avT ; l]       = [V | 1].T @ P^T  (fused unnormalized attention + sum)
      bc              = 1s.T @ (1/l)     (C=1 PE broadcast matmul, fp32r)
      Z               = avT * bc         (DVE)
      out_partial     = Z.T @ WoT_slice  (PSUM -> SBUF fp16 -> DRAM)

  Scheduling (the Tile scheduler orders each engine's queue by emission
  priority among dependency-ready instructions, so program order is the
  pacing tool):
    - the ACT exp stream (~1.11us per t-chunk pair) and the PE are both
      near-saturated mid-tile; all projection / out-projection work is
      split into <=4-matmul work items in a build-time queue, popped 2-3
      per t-iteration so it interleaves with (never preempts) the
      exp-critical score matmuls;
    - each s-tile's last 3 AV pairs + PSUM drain (z copies, 1/l, DRAM-
      bounce broadcast, normalize) are carried into the next s-tile's
      first iterations, so s-tile boundaries stay seamless;
    - out-projection of s-tile n is deferred into s-tile n+1 (hides the
      broadcast latency); the final s-tile normalizes via a C=1 PE
      broadcast matmul instead of the DRAM bounce (shorter kernel tail);
    - batch 0 loads x before weights (bias DMAs first), warms the PE HAM
      clock with junk matmuls during the x DMA, and feeds K/Q for s-tile 0
      only, so the exp stream starts ~21us in; junk matmuls at batch seams
      keep the clock at 2.4GHz through the drain chains.
  Baseline 465.6us -> ~414-422us measured (rel err 5.6e-4); each batch carries its own V-tail (V st3 + trans 8-15) in its first s-tile rather than crowding the previous batch's window.
"""

import numpy as np

import concourse.bass as bass
import concourse.mybir as mybir
import concourse.tile as tile
from concourse import bacc
from concourse.bass_utils import run_bass_kernel_spmd
from concourse.masks import make_identity

# Problem shapes (hardcoded per contract).
B, S, D = 4, 2048, 1024
H, E = 16, 64
NCORES = 8
HPC = H // NCORES          # heads per core = 2
EC = HPC * E               # per-core head width = 128
BS = B * S                 # 8192 rows
P = 128
DC = D // P                # 8 contraction chunks for the projections
DC2 = DC // 2              # 4 double-row chain steps
ST = 512                   # s tile (matmul moving free dim)
N_ST = S // ST             # 4 s-tiles per batch
TCH = S // P               # 16 key chunks per batch

F32 = mybir.dt.float32
F32R = mybir.dt.float32r
F16 = mybir.dt.float16
F8 = mybir.dt.float8e4
DR = mybir.MatmulPerfMode.DoubleRow
EXP = mybir.ActivationFunctionType.Exp

# Host-side scaling: weights are multiplied by WSCALE before the fp8e4 cast
# so their magnitude (~1/32) lands in e4m3's sweet spot.  Q and K then carry
# WSCALE each, so the exp argument is scores/(8*WSCALE^2); V carries WSCALE,
# compensated by dividing Wo by WSCALE on the host.  The exp bias -ln(32)
# keeps the unnormalized z accumulation inside fp16 range (it cancels in the
# softmax normalization).
WSCALE = 32.0
EXP_SCALE = 0.125 / (WSCALE * WSCALE)
EXP_BIAS = -3.4657359027997265  # -ln(32)


def _r(ap):
    return ap.bitcast(F32R)


def build_module():
    """Build the single-core Bass module (same NEFF runs SPMD on all 8 cores)."""
    from contextlib import ExitStack

    nc = bacc.Bacc("TRN2", target_bir_lowering=False, debug=False)
    xt = nc.dram_tensor("xt", [D, BS], F16, kind="ExternalInput").ap()
    wq = nc.dram_tensor("wq_t", [D, EC], F16, kind="ExternalInput").ap()
    wk = nc.dram_tensor("wk_t", [D, EC], F16, kind="ExternalInput").ap()
    wv = nc.dram_tensor("wv_t", [D, EC], F16, kind="ExternalInput").ap()
    bq = nc.dram_tensor("bq", [EC, 1], F32, kind="ExternalInput").ap()
    bk = nc.dram_tensor("bk", [EC, 1], F32, kind="ExternalInput").ap()
    bv = nc.dram_tensor("bv", [EC, 1], F32, kind="ExternalInput").ap()
    wo = nc.dram_tensor("wo_t", [EC, D], F16, kind="ExternalInput").ap()
    outp = nc.dram_tensor("out_p", [BS, D], F16, kind="ExternalOutput").ap()

    xt_r = xt.rearrange("(dc p) s -> p dc s", p=P)    # [128, 8, 8192]
    wq_r = wq.rearrange("(dc p) e -> p dc e", p=P)    # [128, 8, 128]
    wk_r = wk.rearrange("(dc p) e -> p dc e", p=P)
    wv_r = wv.rearrange("(dc p) e -> p dc e", p=P)

    with tile.TileContext(nc) as tc, ExitStack() as ctx:
        singles = ctx.enter_context(tc.tile_pool(name="singles", bufs=1))

        wq_sb = singles.tile([P, DC, EC], F16, tag="wq")
        wk_sb = singles.tile([P, DC, EC], F16, tag="wk")
        wv_sb = singles.tile([P, DC, EC], F16, tag="wv")
        bq_sb = singles.tile([EC, 1], F32, tag="bq")
        bk_sb = singles.tile([EC, 1], F32, tag="bk")
        bv_sb = singles.tile([EC, 1], F32, tag="bv")
        wo_sb = singles.tile([EC, D], F16, tag="wo")
        ident = singles.tile([P, P], F16, tag="ident")
        make_identity(nc, ident[:])
        ones16 = singles.tile([1, E], F16, tag="ones16")
        nc.vector.memset(ones16[:], 1.0)
        ebias = singles.tile([P, 1], F32, tag="ebias")
        nc.vector.memset(ebias[:], EXP_BIAS)

        # Per-batch persistent activations: [e, s] projections and V_ext.
        qt = [singles.tile([EC, S], F16, tag=f"qt{b}", name=f"qt{b}") for b in range(B)]
        kt = [singles.tile([EC, S], F16, tag=f"kt{b}", name=f"kt{b}") for b in range(B)]
        vt = [singles.tile([EC, S], F16, tag=f"vtz{b}", name=f"vt{b}") for b in range(B)]
        # V_ext layout: [t-part, t-chunk, 130] = [V_h0 | 1 | V_h1 | 1]
        vx = [singles.tile([P, TCH, 2 * E + 2], F16, tag=f"vx{b}", name=f"vx{b}") for b in range(B)]
        for b in range(B):
            nc.vector.memset(vx[b][:, :, E : E + 1], 1.0)
            nc.vector.memset(vx[b][:, :, 2 * E + 1 : 2 * E + 2], 1.0)

        z = [singles.tile([EC, S], F16, tag=f"z{b}", name=f"z{b}") for b in range(B)]
        xts = [[None] * N_ST for _ in range(B)]
        CRIT = -2000000  # junk clock-warmers: run only when PE is otherwise idle
        # Priority classes for queued bulk work.  The Tile scheduler is a
        # timing simulation that, whenever an engine goes idle, runs the
        # highest-priority *dependency-ready* instruction.  Emission order
        # still defines the dependency DAG (tile-slot reuse), but by emitting
        # bulk work at low priority the critical score->exp->AV stream jumps
        # ahead of any backlog the moment its own inputs are ready, instead
        # of queueing behind 4+ projection chains at every s-tile boundary.
        # Deadline-ordered tiers: next-batch K/V/x/transposes are ALL
        # consumed within that batch's FIRST s-tile, so they are due at the
        # seam; the out-projection has multiple tiles of staging slack.
        SOON = 300000    # qt next tile, V tail, and next-batch K/V/x/trans
        BULK = 450000    # out-projection (runs in later PE gaps)
        PBIAS = 250000   # proj bias-adds: they release shared PSUM slots, so
                         # they outrank all queued bulk work on the DVE heap

        with (
            tc.tile_pool(name="xload", bufs=6) as xpool,
            tc.tile_pool(name="pexp", bufs=14) as ppool,
            tc.tile_pool(name="bcast", bufs=3) as bpool,
            tc.tile_pool(name="ostage", bufs=6) as opool,
            tc.tile_pool(name="lrow", bufs=2, space="DRAM") as dpool,
            tc.tile_pool(name="psum", bufs=2, space="PSUM") as psum,
            tc.tile_pool(name="psum_av", bufs=1, space="PSUM") as psum_av,
        ):

            from collections import deque

            gap_q = deque()  # entries: (prio, seq, fn); FIFO mid-tile pops
            gap_seq = [0]

            def gap_push(prio, fn):
                gap_q.append((prio, gap_seq[0], fn))
                gap_seq[0] += 1

            def pop_gap():
                prio, _, fn = gap_q.popleft()
                with tc.high_priority(offset=-prio):
                    fn()

            def drain_gap_by_prio():
                # At s-tile entry the whole backlog must be emitted before the
                # tile's scores (emission order defines the dependency DAG).
                # Emit it deadline-first: the PSUM pool slot rotation is fixed
                # by emission order, so this keeps seam-critical K/V chains
                # from queueing behind out-projection slots that are held by
                # slow DVE casts.
                for prio, _, fn in sorted(gap_q, key=lambda e: (e[0], e[1])):
                    with tc.high_priority(offset=-prio):
                        fn()
                gap_q.clear()

            def _bulk(fn):
                with tc.high_priority(offset=-SOON):
                    fn()

            def emit_x(b, sts=range(N_ST), fine=False):
                # split each 1MB tile into 4 DMAs so they spread across
                # queues; `fine` splits per dc-chunk so the batch-0 head's
                # K projection chain can start on partial data
                for st in sts:
                    g = b * N_ST + st
                    x_t = xpool.tile([P, DC, ST], F16, tag="xt", name="x_t")
                    step = 1 if fine else 2
                    for hh in range(DC // step):
                        nc.sync.dma_start(
                            x_t[:, step * hh : step * hh + step],
                            xt_r[:, step * hh : step * hh + step,
                                 g * ST : (g + 1) * ST],
                        )
                    xts[b][st] = x_t

            def emit_proj(b, st, w_sb, b_sb, dst):
                # One 512-wide projection chain: dst[:, st] = W @ X^T + bias
                sl = slice(st * ST, (st + 1) * ST)
                ps = psum.tile([P, ST], F32, tag="mm", name="ps")
                for dc in range(DC):
                    nc.tensor.matmul(
                        ps[:], w_sb[:, dc], xts[b][st][:, dc],
                        start=(dc == 0), stop=(dc == DC - 1),
                    )
                nc.vector.tensor_scalar_add(dst[:, sl], ps[:], b_sb[:])

            def push_proj(b, st, w_sb, b_sb, dst, prio=SOON):
                # Same chain, split into two 2-matmul work items: small enough
                # to interleave with the exp stream, big enough to keep the
                # PSUM-slot hold time short.
                sl = slice(st * ST, (st + 1) * ST)
                box = {}

                def item(i):
                    def go():
                        if i == 0:
                            box["ps"] = psum.tile([P, ST], F32, tag="mm", name="ps")
                        ps = box["ps"]
                        for dc in range(4 * i, 4 * i + 4):
                            nc.tensor.matmul(
                                ps[:], w_sb[:, dc], xts[b][st][:, dc],
                                start=(dc == 0), stop=(dc == DC - 1),
                            )
                        if i == 1:
                            with tc.high_priority(offset=prio - PBIAS):
                                nc.vector.tensor_scalar_add(dst[:, sl], ps[:], b_sb[:])
                    return go

                for i in range(2):
                    gap_push(prio, item(i))

            def emit_trans(b, chunks):
                # PE-transpose batch b's V chunks into V_ext.
                for tch in chunks:
                    tp = psum.tile([P, ST], F16, tag="mm", name="tp")
                    nc.tensor.transpose(
                        tp[:, 0:P], vt[b][:, tch * P : (tch + 1) * P], ident[:]
                    )
                    nc.vector.tensor_copy(vx[b][:, tch, 0:E], tp[:, 0:E])
                    nc.vector.tensor_copy(
                        vx[b][:, tch, E + 1 : 2 * E + 1], tp[:, E : 2 * E]
                    )

            def push_trans(b, lo, hi, prio=SOON):
                for c0 in range(lo, hi, 2):
                    gap_push(prio, lambda c0=c0: emit_trans(b, range(c0, c0 + 2)))

            def push_outproj(b, st, final=False):
                for c in range(ST // P):
                    def go(c=c):
                        zsl = slice(st * ST + c * P, st * ST + (c + 1) * P)
                        rows = slice(b * S + st * ST + c * P,
                                     b * S + st * ST + (c + 1) * P)
                        for oh in range(D // 512):
                            po = psum.tile([P, ST], F32, tag="mm", name="po")
                            nc.tensor.matmul(
                                po[:], z[b][:, zsl],
                                wo_sb[:, oh * 512 : (oh + 1) * 512],
                                start=True, stop=True,
                            )
                            osb = opool.tile([P, 512], F16, tag="osb", name="osb")
                            if final and (c + oh) % 2 == 1:
                                # kernel tail: ACT is idle, split the casts
                                # across both engines to drain PSUM 2x faster
                                nc.scalar.copy(osb[:], po[:])
                            else:
                                nc.vector.tensor_copy(osb[:], po[:])
                            nc.sync.dma_start(
                                outp[rows, oh * 512 : (oh + 1) * 512], osb[:])
                    gap_push(BULK, go)

            pending = [None]
            carrybox = {"c": []}
            AVLAG = 3

            def emit_attn(b, st, hooks):
                # batch seams tend to idle the PE >3.4us on the DVE drain
                # chain, dropping HAM to 1.2GHz right before a burst of
                # deferred chains — junk matmuls keep the clock up.
                if st == 0 and b > 0:
                    with tc.high_priority(offset=CRIT):
                        for _ in range(10):
                            nc.tensor.matmul(wu[:, 0:P], ident[:], ident[:],
                                             start=True, stop=True)
                # safety: queued producers for this s-tile's inputs must be
                # EMITTED before its consumers (emission order defines the
                # dependency DAG).  They are emitted at their low priority, so
                # this s-tile's scores still run ahead of any backlog that
                # this tile's inputs don't actually depend on.
                drain_gap_by_prio()
                ssl = slice(st * ST, (st + 1) * ST)
                av = psum_av.tile([P, 2, ST], F32, tag="av", name="av")
                av0 = av[:, 0]
                av1 = av[:, 1]
                pts = {}

                def av_pair(t):
                    p = pts.pop(t)
                    nc.tensor.matmul(
                        av0[0 : E + 1], vx[b][:, t, 0 : E + 1], p[:, 0],
                        start=(t == 0), stop=(t == TCH - 1),
                    )
                    nc.tensor.matmul(
                        av1[0 : E + 1], vx[b][:, t, E + 1 : 2 * E + 2], p[:, 1],
                        start=(t == 0), stop=(t == TCH - 1),
                    )

                def drain():
                    # unnormalized copy out of PSUM (frees the av banks),
                    # reciprocal of the fused row-sums, broadcast, normalize.
                    # The l rows are copied to TWO partitions so the serial
                    # reciprocal runs at FD=512 instead of FD=1024.
                    nc.vector.tensor_copy(z[b][0:E, ssl], av0[0:E])
                    nc.vector.tensor_copy(z[b][E : 2 * E, ssl], av1[0:E])
                    lr = bpool.tile([1, 2, ST], F32, tag="lr", name="lr")
                    nc.vector.tensor_copy(lr[0:1], av[E : E + 1, :, :])
                    nc.vector.reciprocal_approx_fast(out=lr[0:1], in_=lr[0:1])
                    if (b, st) == (B - 1, N_ST - 1):
                        # final s-tile: a DRAM bounce would sit on the kernel
                        # tail; use C=1 PE broadcast matmuls instead (moving
                        # must sit at partition 0, so cast each head's row
                        # down separately).
                        lr16a = bpool.tile([1, ST], F16, tag="lr16a", name="lr16a")
                        lr16b = bpool.tile([1, ST], F16, tag="lr16b", name="lr16b")
                        nc.vector.tensor_copy(lr16a[0:1], lr[0:1, 0, :])
                        nc.vector.tensor_copy(lr16b[0:1], lr[0:1, 1, :])
                        bc0 = psum.tile([E, ST], F32, tag="mm", name="bc0")
                        nc.tensor.matmul(bc0[:], ones16[0:1, :], lr16a[0:1, :],
                                         start=True, stop=True)
                        nc.vector.tensor_mul(z[b][0:E, ssl], z[b][0:E, ssl], bc0[:])
                        bc1 = psum.tile([E, ST], F32, tag="mm", name="bc1")
                        nc.tensor.matmul(bc1[:], ones16[0:1, :], lr16b[0:1, :],
                                         start=True, stop=True)
                        nc.vector.tensor_mul(
                            z[b][E : 2 * E, ssl], z[b][E : 2 * E, ssl], bc1[:])
                    else:
                        lrow = dpool.tile([2, ST], F32, tag="lrow", name="lrow")
                        nc.sync.dma_start(
                            bass.AP(tensor=lrow.tensor, offset=lrow.offset,
                                    ap=[[0, 1]] + list(lrow.ap)),
                            lr[0:1, :, :],
                        )
                        bc = bpool.tile([P, ST], F32, tag="bc", name="bc")
                        nc.sync.dma_start(
                            bc[0:E],
                            bass.AP(tensor=lrow.tensor, offset=lrow.offset,
                                    ap=[[0, E]] + list(lrow[0, :].ap)),
                        )
                        nc.sync.dma_start(
                            bc[E : 2 * E],
                            bass.AP(tensor=lrow.tensor, offset=lrow.offset + ST,
                                    ap=[[0, E]] + list(lrow[1, :].ap)),
                        )
                        nc.vector.tensor_mul(
                            z[b][0:E, ssl], z[b][0:E, ssl], bc[0:E])
                        nc.vector.tensor_mul(
                            z[b][E : 2 * E, ssl], z[b][E : 2 * E, ssl], bc[E : 2 * E]
                        )

                prev = carrybox["c"]
                for t in range(TCH):
                    tsl = slice(t * P, (t + 1) * P)
                    sc = psum.tile([P, 2, ST], F32, tag="sc", name="sc")
                    nc.tensor.matmul(
                        sc[:, 0], kt[b][0:E, tsl], qt[b][0:E, ssl],
                        start=True, stop=True,
                    )
                    nc.tensor.matmul(
                        sc[:, 1], kt[b][E : 2 * E, tsl], qt[b][E : 2 * E, ssl],
                        start=True, stop=True,
                    )
                    pt = ppool.tile([P, 2, ST], F16, tag="pt", name="pt")
                    pts[t] = pt
                    nc.scalar.activation(pt[:], sc[:], EXP, scale=0.125)
                    for fn in hooks.get(t, ()):
                        fn()
                    if t == 4 and pending[0] is not None:
                        push_outproj(*pending[0])
                        pending[0] = None
                    # previous s-tile's last av pairs + drain, carried into
                    # this s-tile's first iters (keeps boundaries seamless)
                    if t < len(prev):
                        prev[t]()
                    if t >= AVLAG:
                        av_pair(t - AVLAG)
                    for _ in range(3 if len(gap_q) > 12 else 2):
                        if gap_q:
                            pop_gap()
                carrybox["c"] = [
                    lambda: av_pair(TCH - 3),
                    lambda: av_pair(TCH - 2),
                    lambda: (av_pair(TCH - 1), drain()),
                ]
                pending[0] = (b, st)

            # ---- push schedules: all projection / transpose / prefetch work
            # for the next batch rides the work queue, paced 2 items/iter.
            # Producers must be QUEUED before their consumers are emitted
            # (program order defines dependencies).
            def hooks_for(b, st):
                hk = {}

                def add(t, fn):
                    hk.setdefault(t, []).append(fn)

                if st + 1 < N_ST:
                    add(0, lambda: push_proj(b, st + 1, wq_sb, bq_sb, qt[b], prio=SOON))
                if st == 0:
                    # this batch's own V tail: consumed only at iters 11+,
                    # so it can ride this batch's own first s-tile instead
                    # of crowding the previous batch's window.
                    add(0, lambda: push_proj(b, 3, wv_sb, bv_sb, vt[b], prio=SOON))
                    add(2, lambda: push_trans(b, 8, 12, prio=SOON))
                    add(4, lambda: push_trans(b, 12, 16, prio=SOON))
                nb = b + 1
                if nb < B and b > 0:
                    if st == 0:
                        add(6, lambda: _bulk(lambda: emit_x(nb)))
                        add(8, lambda: push_proj(nb, 0, wk_sb, bk_sb, kt[nb]))
                        add(12, lambda: push_proj(nb, 1, wk_sb, bk_sb, kt[nb]))
                    elif st == 1:
                        add(4, lambda: push_proj(nb, 2, wk_sb, bk_sb, kt[nb]))
                        add(8, lambda: push_proj(nb, 3, wk_sb, bk_sb, kt[nb]))
                        add(12, lambda: push_proj(nb, 0, wv_sb, bv_sb, vt[nb]))
                    elif st == 2:
                        add(2, lambda: push_proj(nb, 1, wv_sb, bv_sb, vt[nb]))
                        add(6, lambda: push_trans(nb, 0, 4))
                        add(10, lambda: push_trans(nb, 4, 8))
                    else:
                        add(2, lambda: push_proj(nb, 0, wq_sb, bq_sb, qt[nb]))
                        add(6, lambda: push_proj(nb, 2, wv_sb, bv_sb, vt[nb]))
                return hk

            # ---- batch 0 head: tiny bias/weight DMAs first, x split across
            # queues, a PE warmup burst (flips HAM to 2.4GHz), then just
            # K0/Q0 so the exp stream starts ASAP.
            nc.sync.dma_start(bq_sb[:], bq)
            nc.sync.dma_start(bk_sb[:], bk)
            nc.sync.dma_start(bv_sb[:], bv)
            nc.sync.dma_start(wk_sb[:], wk_r)
            nc.sync.dma_start(wq_sb[:], wq_r)
            emit_x(0, sts=[0])
            nc.sync.dma_start(wv_sb[:], wv_r)
            nc.sync.dma_start(wo_sb[:], wo)
            emit_x(0, sts=[1, 2, 3])
            with tc.high_priority(offset=CRIT):
                wu = psum.tile([P, ST], F32, tag="mm", name="wu")
                for _ in range(28):
                    nc.tensor.matmul(wu[:, 0:P], ident[:], ident[:],
                                     start=True, stop=True)
            emit_proj(0, 0, wk_sb, bk_sb, kt[0])
            emit_proj(0, 0, wq_sb, bq_sb, qt[0])

            # batch-0 st0: V chunk 0-3 path emitted directly (av(0) needs it
            # almost immediately); the rest is queued in deadline order.
            b0_hooks = [
                {
                    0: [lambda: emit_proj(0, 0, wv_sb, bv_sb, vt[0]),
                        lambda: emit_trans(0, range(0, 2)),
                        lambda: push_proj(0, 1, wk_sb, bk_sb, kt[0], prio=SOON)],
                    1: [lambda: emit_trans(0, range(2, 4)),
                        lambda: push_proj(0, 1, wv_sb, bv_sb, vt[0], prio=SOON)],
                    2: [lambda: push_trans(0, 4, 8, prio=SOON),
                        lambda: push_proj(0, 2, wk_sb, bk_sb, kt[0], prio=SOON)],
                    5: [lambda: push_proj(0, 2, wv_sb, bv_sb, vt[0], prio=SOON),
                        lambda: push_trans(0, 8, 12, prio=SOON)],
                    7: [lambda: push_proj(0, 3, wk_sb, bk_sb, kt[0], prio=SOON)],
                    9: [lambda: push_proj(0, 3, wv_sb, bv_sb, vt[0], prio=SOON),
                        lambda: push_trans(0, 12, 16, prio=SOON)],
                    11: [lambda: push_proj(0, 1, wq_sb, bq_sb, qt[0], prio=SOON)],
                    13: [lambda: _bulk(lambda: emit_x(1, sts=[0, 1]))],
                },
                {
                    0: [lambda: push_proj(0, 2, wq_sb, bq_sb, qt[0], prio=SOON)],
                    2: [lambda: _bulk(lambda: emit_x(1, sts=[2, 3]))],
                    8: [lambda: push_proj(1, 0, wk_sb, bk_sb, kt[1])],
                    12: [lambda: push_proj(1, 1, wk_sb, bk_sb, kt[1])],
                },
                {
                    0: [lambda: push_proj(0, 3, wq_sb, bq_sb, qt[0], prio=SOON)],
                    4: [lambda: push_proj(1, 2, wk_sb, bk_sb, kt[1])],
                    8: [lambda: push_proj(1, 3, wk_sb, bk_sb, kt[1])],
                    12: [lambda: push_proj(1, 0, wv_sb, bv_sb, vt[1])],
                },
                {
                    2: [lambda: push_proj(1, 1, wv_sb, bv_sb, vt[1])],
                    6: [lambda: push_trans(1, 0, 4)],
                    8: [lambda: push_proj(1, 0, wq_sb, bq_sb, qt[1])],
                    10: [lambda: push_proj(1, 2, wv_sb, bv_sb, vt[1])],
                    12: [lambda: push_trans(1, 4, 8)],
                },
            ]
            for st in range(N_ST):
                emit_attn(0, st, b0_hooks[st])

            for b in range(1, B):
                for st in range(N_ST):
                    emit_attn(b, st, hooks_for(b, st))

            # tail: leftover carry (last av pairs + final drain), remaining
            # queue, and the final s-tile's out-projection. Junk matmuls keep
            # the PE HAM-warm through the serial drain chain so the final
            # out-projection runs at 2.4GHz.
            for fn in carrybox["c"]:
                fn()
            with tc.high_priority(offset=CRIT):
                for _ in range(8):
                    nc.tensor.matmul(wu[:, 0:P], ident[:], ident[:],
                                     start=True, stop=True)
            while gap_q:
                pop_gap()
            push_outproj(*pending[0], final=True)
            while gap_q:
                pop_gap()
    nc.finalize()
    return nc


_NC_CACHE = None


def _get_module():
    global _NC_CACHE
    if _NC_CACHE is None:
        _NC_CACHE = build_module()
    return _NC_CACHE


def prepare_in_maps(inputs):
    x = np.ascontiguousarray(np.asarray(inputs["input_matrix"], np.float32))
    wq = np.asarray(inputs["Wq"], np.float32)
    wk = np.asarray(inputs["Wk"], np.float32)
    wv = np.asarray(inputs["Wv"], np.float32)
    bq = np.asarray(inputs["bq"], np.float32)
    bk = np.asarray(inputs["bk"], np.float32)
    bv = np.asarray(inputs["bv"], np.float32)
    wo = np.asarray(inputs["Wo"], np.float32)

    xt = np.ascontiguousarray(x.reshape(BS, D).T.astype(np.float16))  # [D, BS]
    in_maps = []
    for c in range(NCORES):
        hs = slice(HPC * c, HPC * (c + 1))
        m = {
            "xt": xt,
            "wq_t": np.ascontiguousarray(wq[hs].transpose(2, 0, 1).reshape(D, EC).astype(np.float16)),
            "wk_t": np.ascontiguousarray(wk[hs].transpose(2, 0, 1).reshape(D, EC).astype(np.float16)),
            "wv_t": np.ascontiguousarray(wv[hs].transpose(2, 0, 1).reshape(D, EC).astype(np.float16)),
            "bq": np.ascontiguousarray(bq[hs].reshape(EC, 1)),
            "bk": np.ascontiguousarray(bk[hs].reshape(EC, 1)),
            "bv": np.ascontiguousarray(bv[hs].reshape(EC, 1)),
            "wo_t": np.ascontiguousarray(wo[:, EC * c : EC * (c + 1)].T.astype(np.float16)),
        }
        in_maps.append(m)
    return in_maps


def finish(results, inputs):
    bo = np.asarray(inputs["bo"], np.float32)
    acc = results[0]["out_p"].astype(np.float32)
    for r in results[1:]:
        acc += r["out_p"].astype(np.float32)
    out = (acc + bo).astype(np.float32)
    return out.reshape(B, S, D)


def kernel(**inputs):
    nc = _get_module()
    in_maps = prepare_in_maps(inputs)
    res = run_bass_kernel_spmd(nc, in_maps, core_ids=list(range(NCORES)))
    return finish(res.results, inputs)


if __name__ == "__main__":
    import reference

    inputs = {k: np.asarray(v) for k, v in reference.setup_inputs().items()}
    out = kernel(**inputs)
    print(out.shape, out.dtype)

